# revision 17
# baseline (speedup 1.0000x reference)
"""Multi-head self-attention block (B=2, N=2048, C=1024, H=16, D=64) + output
projection, sharded over 8 Trainium2 NeuronCores.

Sharding: core c handles batch b=c//4 and heads 4*(c%4)..4*(c%4)+3 (data +
head parallel).  The output projection is row-sharded over the input-channel
dim (each core multiplies its 256 attention channels into a full [N, 1024]
partial product); the 4 partials per batch are summed on the host (the
"all-reduce") and the bias is added there.

Device kernel layout (per core):
  - q, k are fed pre-transposed per head-pair as fp16: [128, N] tiles whose
    partition dim stacks the two heads' 64 attention dims.  fp16 streams at
    1 cyc/col on the PE (f32r row_grp=h0 measured 1.5 cyc/col) and halves
    the q/k input DMA.
  - scores_T[k_row, n] for a 128-row key chunk come from one K=64 matmul per
    head (the two heads use disjoint PE row groups).
  - exp() is split across TWO engines: the Scalar engine (table Exp,
    PSUM -> SBUF bf16) and the Vector engine (one-pass Schraudolph bit-trick:
    int16(x*2^7*log2e + (127*2^7 - C)) reinterpreted as bf16, ~3% rel err on
    a configurable fraction of tiles).  Baseline was Scalar-only and exp
    paced the whole attention phase (~1114ns/tile vs ~970ns PE work).
  - AV: lhsT is v augmented with a ones column, so PSUM accumulates x^T
    unnormalized (rows 0-63 / 64-127) and the softmax denominator (row 64 for
    even heads, row 32 for odd heads) in the same accumulation group.
  - normalization: denominator rows go psum->DRAM->sbuf via DMA, one cheap
    reciprocal on [2, HW], DMA row-broadcast back, and the normalize multiply
    is fused into the PSUM evacuation (vector does rows 0:64, gpsimd rows
    64:128) writing the f16 projection lhsT directly.
  - projection: emitted per query-half right after that half's attention so
    output DMA overlaps the second half; [N,256]@[256,1024] partial product
    written unreduced (bf16 by default, summed as f32 on host).
"""

import os
from contextlib import ExitStack

import ml_dtypes
import numpy as np

import concourse.bass as bass
import concourse.tile as tile
from concourse import bacc, mybir
from concourse._compat import with_exitstack
from concourse import bass_utils

F32 = mybir.dt.float32

B, N, C, H, D = 2, 2048, 1024, 16, 64
NCORES = 8
HPC = 4  # heads per core
NPAIR = HPC // 2

# Schraudolph bf16 exp constants: i16 = round(x * 2^7/ln2 + (127*2^7 - C));
# C=7.4 minimizes mean rel err (~1.5%, max ~4.2%) over the scores range.
EXP_A = 2.0**7 / float(np.log(2.0))
EXP_B = 127.0 * 2.0**7 - 7.4


def _mm_dtypes():
    """PE dtypes for the three matmul groups (all 1 cyc/col on TRN2)."""
    qk = os.environ.get("ATTN_KERNEL_QK_DT", "f16")
    av = os.environ.get("ATTN_KERNEL_AV_DT", "bf16")
    pj = os.environ.get("ATTN_KERNEL_PJ_DT", "f16")
    m = {
        "f32": F32,
        "f32r": mybir.dt.float32r,
        "bf16": mybir.dt.bfloat16,
        "f16": mybir.dt.float16,
    }
    return m[qk], m[av], m[pj]


def _out_dtype():
    return {"f32": F32, "bf16": mybir.dt.bfloat16}[
        os.environ.get("ATTN_KERNEL_OUT_DT", "bf16")
    ]


def _dve_exp_ratio():
    """(num, den): of every `den` exp tiles, `num` go to the Vector engine."""
    s = os.environ.get("ATTN_KERNEL_DVE_EXP", "9:16")
    if ":" in s:
        a, b = s.split(":")
        return int(a), int(b)
    return int(s), 2


def _bcast_row(row_ap, nparts):
    """DRAM AP view replicating a 1D row across `nparts` partitions."""
    return bass.AP(
        tensor=row_ap.tensor,
        offset=row_ap.offset,
        ap=[[0, nparts], *row_ap.ap],
    )


@with_exitstack
def attention_body(ctx: ExitStack, tc: tile.TileContext, out, qt, kt, vp, wt):
    """Emit the per-core attention+projection program.

    APs:
      out  [N, OW]          partial projection output
      qt   [NPAIR, 128, N]  q transposed, head pair stacked on partitions
      kt   [NPAIR, 128, N]  k transposed, same packing
      vp   [2*NPAIR, 128, NJ, 128]  v chunks as AV lhsT: for even heads v in
           cols 0:64 and ones in col 64; for odd heads v in cols 64:128 and
           ones in col 32 (so x^T lands on the partitions matching qt packing)
      wt   [NPAIR, 128, OW] proj_w slice, transposed to [channel, out]
    """
    nc = tc.nc
    P = 128
    npair, _, n = qt.shape
    NJ = n // P          # key chunks
    HW = n // 2          # query half processed per inner loop
    NT = max(1, HW // 512)
    MS = HW // NT        # matmul free-dim chunk (<=512)
    OW = wt.shape[2]
    OT = max(1, OW // 512)
    OS = OW // OT
    dve_num, dve_den = _dve_exp_ratio()

    sing = ctx.enter_context(tc.tile_pool(name="sing", bufs=1))
    probs_pool = ctx.enter_context(tc.tile_pool(name="probs", bufs=4))
    work = ctx.enter_context(tc.tile_pool(name="work", bufs=2))
    ost = ctx.enter_context(tc.tile_pool(name="ost", bufs=3))
    psum = ctx.enter_context(tc.tile_pool(name="psum", bufs=2, space="PSUM"))
    dram = ctx.enter_context(tc.tile_pool(name="dram", bufs=2, space="DRAM"))

    # HAM warm-up: dense plain-fp32 matmuls on a constant tile run during the
    # input DMA window (no data dependency) and lift the PE p-state before the
    # real f16/bf16 stream begins
    nwarm = int(os.environ.get("ATTN_KERNEL_WARMUP", "6"))
    nburst = int(os.environ.get("ATTN_KERNEL_REWARM", "1"))
    wtile = None
    if nwarm or nburst:
        wtile = sing.tile([P, 512], F32, tag="warm", name="warm")
        nc.vector.memset(wtile, 1.0)

    def warm_burst(count, name):
        pw = psum.tile([P, 512], F32, tag="ps", bufs=4, name=name)
        for w in range(count):
            nc.tensor.matmul(
                pw, lhsT=wtile[:, 0:128], rhs=wtile, start=True, stop=True
            )

    if nwarm:
        warm_burst(nwarm, "warmps")

    # Input DMA order matters: the first AV (vp) fires ~3 chunks into the
    # first unit, so vp[0..1] must land right after qt0/kt0 -- before qt1/kt1
    # and long before wt (only needed at the projection).
    qts, kts, wts, xts, vps = [], [], [], [], []
    for p in range(npair):
        qts.append(sing.tile([P, n], qt.dtype, tag=f"qt{p}", name=f"qts{p}"))
        kts.append(sing.tile([P, n], kt.dtype, tag=f"kt{p}", name=f"kts{p}"))
        wts.append(sing.tile([P, OW], wt.dtype, tag=f"wt{p}", name=f"wts{p}"))
        xts.append(sing.tile([P, n], wt.dtype, tag=f"xt{p}", name=f"xts{p}"))
    for h in range(2 * npair):
        vps.append(sing.tile([P, NJ, P], vp.dtype, tag=f"vp{h}", name=f"vps{h}"))
    for p in range(npair):
        nc.sync.dma_start(qts[p], qt[p])
        nc.sync.dma_start(kts[p], kt[p])
        nc.sync.dma_start(vps[2 * p], vp[2 * p])
        nc.sync.dma_start(vps[2 * p + 1], vp[2 * p + 1])
    for p in range(npair):
        nc.sync.dma_start(wts[p], wt[p])

    def proj_tiles(lo, hi):
        # projection for query rows [lo*128, hi*128): emitted once both
        # pairs' normalized x^T columns for that range are in flight
        for i in range(lo, hi):
            ot = ost.tile([P, OW], out.dtype, tag="ot", name=f"ot{i}")
            for t in range(OT):
                pp = psum.tile([P, OS], F32, tag="ps", bufs=4, name=f"pp{i}{t}")
                for cc in range(npair):
                    nc.tensor.matmul(
                        pp,
                        lhsT=xts[cc][:, i * P : (i + 1) * P],
                        rhs=wts[cc][:, t * OS : (t + 1) * OS],
                        start=(cc == 0),
                        stop=(cc == npair - 1),
                    )
                if (i + t) % 2 == 0:
                    nc.scalar.copy(ot[:, t * OS : (t + 1) * OS], pp)
                else:
                    nc.vector.tensor_copy(ot[:, t * OS : (t + 1) * OS], pp)
            nc.sync.dma_start(out[i * P : (i + 1) * P, :], ot)

    exp_state = [0]

    def attention_unit(p, q0, qw, un):
        """Attention for head pair p over queries [q0, q0+qw)."""
        NTl = max(1, qw // 512)
        MSl = qw // NTl
        po = [
            psum.tile([P, qw], F32, tag="po", name=f"po{un}{a}")
            for a in range(2)
        ]

        def emit_qk(j):
            # QK for both heads; each (head, free-half) gets its own 1-bank
            # psum tile so 2 chunks of scores can be in flight (bufs=4) --
            # with full-size score tiles the QK(j+1) -> exp(j) -> QK(j)
            # psum-reuse loop paced the whole attention phase.
            pss = [
                [
                    psum.tile(
                        [P, MSl], F32, tag="ps", bufs=4,
                        name=f"ps{un}{j}{a}{t}",
                    )
                    for t in range(NTl)
                ]
                for a in range(2)
            ]
            for t in range(NTl):
                for a in range(2):
                    rows = slice(a * 64, a * 64 + 64)
                    nc.tensor.matmul(
                        pss[a][t],
                        lhsT=kts[p][rows, j * P : (j + 1) * P],
                        rhs=qts[p][rows, q0 + t * MSl : q0 + (t + 1) * MSl],
                        start=True,
                        stop=True,
                    )
            return pss

        # software pipeline: emit QK for chunk j+1 before AV of chunk j so
        # the PE always has ready work behind the exp-gated AV matmuls
        pss = emit_qk(0)
        for j in range(NJ):
            pbs = []
            for a in range(2):
                pb = probs_pool.tile(
                    [P, qw], vp.dtype, tag="pb", name=f"pb{un}{j}{a}"
                )
                for t in range(NTl):
                    sub = pb[:, t * MSl : (t + 1) * MSl]
                    if (exp_state[0] * dve_num) % dve_den < dve_num:
                        # Schraudolph exp on the Vector engine: one mult+add
                        # pass, fp32 PSUM in, int16 store whose bits are the
                        # bf16 probs
                        nc.vector.tensor_scalar(
                            sub.bitcast(mybir.dt.int16),
                            pss[a][t],
                            EXP_A,
                            EXP_B,
                            mybir.AluOpType.mult,
                            mybir.AluOpType.add,
                        )
                    else:
                        nc.scalar.activation(
                            sub, pss[a][t], mybir.ActivationFunctionType.Exp
                        )
                    exp_state[0] += 1
                pbs.append(pb)
            if j + 1 < NJ:
                pss = emit_qk(j + 1)
            for a in range(2):
                for t in range(NTl):
                    nc.tensor.matmul(
                        po[a][:, t * MSl : (t + 1) * MSl],
                        lhsT=vps[2 * p + a][:, j, :],
                        rhs=pbs[a][:, t * MSl : (t + 1) * MSl],
                        start=(j == 0),
                        stop=(j == NJ - 1),
                    )
        # Evacuation: plain Scalar copies free the po banks quickly (the
        # exp engines' in-order queues must never wait on the normalize
        # chain -- a slow op there stalls all subsequent exp and thus AV).
        # Normalization then runs entirely off the exp engines: the
        # denominator rows round-trip through DRAM into a partition-parallel
        # layout, gpsimd computes the final normalize multiplies (SBUF-only).
        dn = work.tile([65, qw], F32, tag="dn", name=f"dn{un}")
        nc.scalar.copy(dn[64:65, :], po[0][64:65, :])
        nc.scalar.copy(dn[32:33, :], po[1][32:33, :])
        xu = work.tile([P, qw], F32, tag="xu", name=f"xu{un}")
        nc.scalar.copy(xu[0:64, :], po[0][0:64, :])
        nc.scalar.copy(xu[64:128, :], po[1][64:128, :])
        dsc = dram.tile([2, qw], F32, tag="dsc", name=f"dsc{un}")
        nc.sync.dma_start(dsc[0:1, :], dn[64:65, :])
        nc.sync.dma_start(dsc[1:2, :], dn[32:33, :])
        G = 2 * qw // P
        dnb = work.tile([P, G], F32, tag="dnb", name=f"dnb{un}")
        nc.sync.dma_start(
            dnb,
            bass.AP(tensor=dsc.tensor, offset=dsc.offset, ap=[[G, P], [1, G]]),
        )
        rcb = work.tile([P, G], F32, tag="rcb", name=f"rcb{un}")
        nc.vector.reciprocal(rcb, dnb)
        dsc2 = dram.tile([2, qw], F32, tag="dsc2", name=f"dsc2{un}")
        nc.sync.dma_start(
            bass.AP(tensor=dsc2.tensor, offset=dsc2.offset,
                    ap=[[G, P], [1, G]]),
            rcb,
        )
        rbd = work.tile([P, qw], F32, tag="rbd", name=f"rbd{un}")
        nc.sync.dma_start(rbd[0:64, :], _bcast_row(dsc2[0], 64))
        nc.sync.dma_start(rbd[64:128, :], _bcast_row(dsc2[1], 64))
        nc.gpsimd.tensor_mul(
            xts[p][0:64, q0 : q0 + qw], xu[0:64, :], rbd[0:64, :]
        )
        nc.gpsimd.tensor_mul(
            xts[p][64:128, q0 : q0 + qw], xu[64:128, :], rbd[64:128, :]
        )
        if nburst:
            warm_burst(nburst, f"rw{un}")

    # Hi=0 as full units; Hi=1 as 512-query sub-units interleaved with the
    # projection so every normalize chain overlaps another unit's attention
    # and the tail after the last unit is short.
    attention_unit(0, 0, HW, "u0")
    attention_unit(1, 0, HW, "u1")
    attention_unit(0, HW, HW // 2, "u2")
    proj_tiles(0, HW // P)
    attention_unit(1, HW, HW // 2, "u3")
    attention_unit(0, HW + HW // 2, HW // 2, "u4")
    proj_tiles(HW // P, HW // P + HW // P // 2)
    attention_unit(1, HW + HW // 2, HW // 2, "u5")
    proj_tiles(HW // P + HW // P // 2, n // P)


def build_module(n=N, ow=C, npair=NPAIR):
    qkd, avd, pjd = _mm_dtypes()
    nc = bacc.Bacc("TRN2", target_bir_lowering=False, debug=False, num_devices=NCORES)
    nj = n // 128
    qt = nc.dram_tensor("qt", [npair, 128, n], qkd, kind="ExternalInput")
    kt = nc.dram_tensor("kt", [npair, 128, n], qkd, kind="ExternalInput")
    vp = nc.dram_tensor("vp", [2 * npair, 128, nj, 128], avd, kind="ExternalInput")
    wt = nc.dram_tensor("wt", [npair, 128, ow], pjd, kind="ExternalInput")
    out = nc.dram_tensor("out", [n, ow], _out_dtype(), kind="ExternalOutput")
    with tile.TileContext(nc) as tc:
        attention_body(tc, out.ap(), qt.ap(), kt.ap(), vp.ap(), wt.ap())
    nc.compile()
    return nc


def shard_inputs(q, k, v, proj_w):
    """Build the 8 per-core input maps from the full tensors."""
    q = np.asarray(q, dtype=np.float32)
    k = np.asarray(k, dtype=np.float32)
    v = np.asarray(v, dtype=np.float32)
    proj_w = np.asarray(proj_w, dtype=np.float32)
    b_, n_, c_ = q.shape
    h_ = k.shape[1]
    d_ = c_ // h_
    nj = n_ // 128
    _np_dt = {"f32": np.float32, "f32r": np.float32, "bf16": ml_dtypes.bfloat16,
              "f16": np.float16}
    qk_np = _np_dt[os.environ.get("ATTN_KERNEL_QK_DT", "f16")]
    # [B, H, D, N]
    qh = np.ascontiguousarray(
        q.reshape(b_, n_, h_, d_).transpose(0, 2, 3, 1).astype(qk_np)
    )
    kh = np.ascontiguousarray(k.transpose(0, 1, 3, 2).astype(qk_np))
    in_maps = []
    for c in range(NCORES):
        b = c // 4
        hh0 = HPC * (c % 4)
        qt = np.ascontiguousarray(qh[b, hh0 : hh0 + HPC].reshape(NPAIR, 128, n_))
        kt = np.ascontiguousarray(kh[b, hh0 : hh0 + HPC].reshape(NPAIR, 128, n_))
        avd = os.environ.get("ATTN_KERNEL_AV_DT", "bf16")
        vp_np = ml_dtypes.bfloat16 if avd == "bf16" else np.float32
        vp = np.zeros((HPC, 128, nj, 128), vp_np)
        for hh in range(HPC):
            vv = v[b, hh0 + hh].reshape(nj, 128, d_).transpose(1, 0, 2)
            if hh % 2 == 0:
                vp[hh][:, :, 0:64] = vv
                vp[hh][:, :, 64] = 1.0
            else:
                vp[hh][:, :, 64:128] = vv
                vp[hh][:, :, 32] = 1.0
        ch0 = hh0 * d_
        pj_np = _np_dt[os.environ.get("ATTN_KERNEL_PJ_DT", "f16")]
        wt = np.ascontiguousarray(
            proj_w[:, ch0 : ch0 + HPC * d_].T.reshape(NPAIR, 128, c_).astype(pj_np)
        )
        in_maps.append({"qt": qt, "kt": kt, "vp": vp, "wt": wt})
    return in_maps


def reduce_outputs(results, proj_b):
    """Sum the per-core partial projections per batch and add the bias."""
    outs = [np.asarray(r["out"], dtype=np.float32) for r in results]
    full = np.stack(
        [outs[0] + outs[1] + outs[2] + outs[3], outs[4] + outs[5] + outs[6] + outs[7]]
    )
    return (full + np.asarray(proj_b, dtype=np.float32)[None, None, :]).astype(
        np.float32
    )


_NC_CACHE = {}


def _get_module():
    if "nc" not in _NC_CACHE:
        _NC_CACHE["nc"] = build_module()
    return _NC_CACHE["nc"]


def kernel(q, k, v, proj_w, proj_b):
    nc = _get_module()
    in_maps = shard_inputs(q, k, v, proj_w)
    trace = bool(int(os.environ.get("ATTN_KERNEL_TRACE", "0")))
    kwargs = {}
    tmpdir = os.environ.get("ATTN_KERNEL_TMPDIR")
    if trace and tmpdir:
        os.makedirs(tmpdir, exist_ok=True)
        kwargs["tmpdir"] = tmpdir
    res = bass_utils.run_bass_kernel_spmd(
        nc, in_maps, core_ids=list(range(NCORES)), trace=trace, **kwargs
    )
    if trace:
        _NC_CACHE["last_results"] = res
    return reduce_outputs(res.results, proj_b)


# revision 23
# speedup vs baseline: 1.1204x; 1.1204x over previous
"""Multi-head self-attention block (B=2, N=2048, C=1024, H=16, D=64) + output
projection, sharded over 8 Trainium2 NeuronCores.

Sharding: core c handles batch b=c//4 and heads 4*(c%4)..4*(c%4)+3 (data +
head parallel).  The output projection is row-sharded over the input-channel
dim (each core multiplies its 256 attention channels into a full [N, 1024]
partial product); the 4 partials per batch are summed on the host (the
"all-reduce") and the bias is added there.

Device kernel layout (per core):
  - q, k are fed pre-transposed per head-pair as fp16: [128, N] tiles whose
    partition dim stacks the two heads' 64 attention dims.  fp16 streams at
    1 cyc/col on the PE (f32r row_grp=h0 measured 1.5 cyc/col) and halves
    the q/k input DMA.
  - scores_T[k_row, n] for a 128-row key chunk come from one K=64 matmul per
    head (the two heads use disjoint PE row groups).
  - exp() is split across TWO engines: the Scalar engine (table Exp,
    PSUM -> SBUF bf16) and the Vector engine (one-pass Schraudolph bit-trick:
    int16(x*2^7*log2e + (127*2^7 - C)) reinterpreted as bf16, ~3% rel err on
    a configurable fraction of tiles).  Baseline was Scalar-only and exp
    paced the whole attention phase (~1114ns/tile vs ~970ns PE work).
  - AV: lhsT is v augmented with a ones column, so PSUM accumulates x^T
    unnormalized (rows 0-63 / 64-127) and the softmax denominator (row 64 for
    even heads, row 32 for odd heads) in the same accumulation group.
  - normalization: denominator rows go psum->DRAM->sbuf via DMA, one cheap
    reciprocal on [2, HW], DMA row-broadcast back, and the normalize multiply
    is fused into the PSUM evacuation (vector does rows 0:64, gpsimd rows
    64:128) writing the f16 projection lhsT directly.
  - projection: emitted per query-half right after that half's attention so
    output DMA overlaps the second half; [N,256]@[256,1024] partial product
    written unreduced (bf16 by default, summed as f32 on host).
"""

import os
from contextlib import ExitStack

import ml_dtypes
import numpy as np

import concourse.bass as bass
import concourse.tile as tile
from concourse import bacc, mybir
from concourse._compat import with_exitstack
from concourse import bass_utils

F32 = mybir.dt.float32

B, N, C, H, D = 2, 2048, 1024, 16, 64
NCORES = 8
HPC = 4  # heads per core
NPAIR = HPC // 2

# Schraudolph bf16 exp constants: i16 = round(x * 2^7/ln2 + (127*2^7 - C));
# C=7.4 minimizes mean rel err (~1.5%, max ~4.2%) over the scores range.
EXP_A = 2.0**7 / float(np.log(2.0))
EXP_B = 127.0 * 2.0**7 - 7.4


def _mm_dtypes():
    """PE dtypes for the three matmul groups (all 1 cyc/col on TRN2)."""
    qk = os.environ.get("ATTN_KERNEL_QK_DT", "f16")
    av = os.environ.get("ATTN_KERNEL_AV_DT", "bf16")
    pj = os.environ.get("ATTN_KERNEL_PJ_DT", "f16")
    m = {
        "f32": F32,
        "f32r": mybir.dt.float32r,
        "bf16": mybir.dt.bfloat16,
        "f16": mybir.dt.float16,
    }
    return m[qk], m[av], m[pj]


def _out_dtype():
    return {"f32": F32, "bf16": mybir.dt.bfloat16}[
        os.environ.get("ATTN_KERNEL_OUT_DT", "bf16")
    ]


def _dve_exp_ratio():
    """(num, den): of every `den` exp tiles, `num` go to the Vector engine."""
    s = os.environ.get("ATTN_KERNEL_DVE_EXP", "9:16")
    if ":" in s:
        a, b = s.split(":")
        return int(a), int(b)
    return int(s), 2


def _bcast_row(row_ap, nparts):
    """DRAM AP view replicating a 1D row across `nparts` partitions."""
    return bass.AP(
        tensor=row_ap.tensor,
        offset=row_ap.offset,
        ap=[[0, nparts], *row_ap.ap],
    )


@with_exitstack
def attention_body(ctx: ExitStack, tc: tile.TileContext, out, qt, kt, vp, wt):
    """Emit the per-core attention+projection program.

    APs:
      out  [N, OW]          partial projection output
      qt   [NPAIR, 128, N]  q transposed, head pair stacked on partitions
      kt   [NPAIR, 128, N]  k transposed, same packing
      vp   [2*NPAIR, 128, NJ, 128]  v chunks as AV lhsT: for even heads v in
           cols 0:64 and ones in col 64; for odd heads v in cols 64:128 and
           ones in col 32 (so x^T lands on the partitions matching qt packing)
      wt   [NPAIR, 128, OW] proj_w slice, transposed to [channel, out]
    """
    nc = tc.nc
    P = 128
    npair, _, n = qt.shape
    NJ = n // P          # key chunks
    HW = n // 2          # query half processed per inner loop
    NT = max(1, HW // 512)
    MS = HW // NT        # matmul free-dim chunk (<=512)
    OW = wt.shape[2]
    OT = max(1, OW // 512)
    OS = OW // OT
    dve_num, dve_den = _dve_exp_ratio()

    sing = ctx.enter_context(tc.tile_pool(name="sing", bufs=1))
    probs_pool = ctx.enter_context(tc.tile_pool(name="probs", bufs=6))
    work = ctx.enter_context(tc.tile_pool(name="work", bufs=2))
    ost = ctx.enter_context(tc.tile_pool(name="ost", bufs=3))
    psum = ctx.enter_context(tc.tile_pool(name="psum", bufs=2, space="PSUM"))
    dram = ctx.enter_context(tc.tile_pool(name="dram", bufs=2, space="DRAM"))

    # HAM warm-up: dense plain-fp32 matmuls on a constant tile run during the
    # input DMA window (no data dependency) and lift the PE p-state before the
    # real f16/bf16 stream begins
    nwarm = int(os.environ.get("ATTN_KERNEL_WARMUP", "6"))
    nburst = int(os.environ.get("ATTN_KERNEL_REWARM", "1"))
    wtile = None
    if nwarm or nburst:
        wtile = sing.tile([P, 512], F32, tag="warm", name="warm")
        nc.vector.memset(wtile, 1.0)

    def warm_burst(count, name):
        pw = psum.tile([P, 512], F32, tag="ps", bufs=6, name=name)
        for w in range(count):
            nc.tensor.matmul(
                pw, lhsT=wtile[:, 0:128], rhs=wtile, start=True, stop=True
            )

    if nwarm:
        warm_burst(nwarm, "warmps")

    # Input DMA order matters: the first AV (vp) fires ~3 chunks into the
    # first unit, so vp[0..1] must land right after qt0/kt0 -- before qt1/kt1
    # and long before wt (only needed at the projection).
    qts, kts, wts, xts, vps = [], [], [], [], []
    for p in range(npair):
        qts.append(sing.tile([P, n], qt.dtype, tag=f"qt{p}", name=f"qts{p}"))
        kts.append(sing.tile([P, n], kt.dtype, tag=f"kt{p}", name=f"kts{p}"))
        wts.append(sing.tile([P, OW], wt.dtype, tag=f"wt{p}", name=f"wts{p}"))
        xts.append(sing.tile([P, n], wt.dtype, tag=f"xt{p}", name=f"xts{p}"))
    for h in range(2 * npair):
        vps.append(sing.tile([P, NJ, P], vp.dtype, tag=f"vp{h}", name=f"vps{h}"))
    for p in range(npair):
        nc.sync.dma_start(qts[p], qt[p])
        nc.sync.dma_start(kts[p], kt[p])
        nc.sync.dma_start(vps[2 * p], vp[2 * p])
        nc.sync.dma_start(vps[2 * p + 1], vp[2 * p + 1])
    for p in range(npair):
        nc.sync.dma_start(wts[p], wt[p])

    def proj_tiles(lo, hi):
        # projection for query rows [lo*128, hi*128): emitted once both
        # pairs' normalized x^T columns for that range are in flight
        for i in range(lo, hi):
            ot = ost.tile([P, OW], out.dtype, tag="ot", name=f"ot{i}")
            for t in range(OT):
                pp = psum.tile([P, OS], F32, tag="ps", bufs=6, name=f"pp{i}{t}")
                for cc in range(npair):
                    nc.tensor.matmul(
                        pp,
                        lhsT=xts[cc][:, i * P : (i + 1) * P],
                        rhs=wts[cc][:, t * OS : (t + 1) * OS],
                        start=(cc == 0),
                        stop=(cc == npair - 1),
                    )
                if (i + t) % 2 == 0:
                    nc.scalar.copy(ot[:, t * OS : (t + 1) * OS], pp)
                else:
                    nc.vector.tensor_copy(ot[:, t * OS : (t + 1) * OS], pp)
            nc.sync.dma_start(out[i * P : (i + 1) * P, :], ot)

    exp_state = [0]

    def attention_unit(p, q0, qw, un):
        """Attention for head pair p over queries [q0, q0+qw)."""
        NTl = max(1, qw // 512)
        MSl = qw // NTl
        po = [
            psum.tile([P, qw], F32, tag="po", name=f"po{un}{a}")
            for a in range(2)
        ]

        def emit_qk(j):
            # QK for both heads; each (head, free-half) gets its own 1-bank
            # psum tile so 2 chunks of scores can be in flight (bufs=4) --
            # with full-size score tiles the QK(j+1) -> exp(j) -> QK(j)
            # psum-reuse loop paced the whole attention phase.
            pss = [
                [
                    psum.tile(
                        [P, MSl], F32, tag="ps", bufs=6,
                        name=f"ps{un}{j}{a}{t}",
                    )
                    for t in range(NTl)
                ]
                for a in range(2)
            ]
            for t in range(NTl):
                for a in range(2):
                    rows = slice(a * 64, a * 64 + 64)
                    nc.tensor.matmul(
                        pss[a][t],
                        lhsT=kts[p][rows, j * P : (j + 1) * P],
                        rhs=qts[p][rows, q0 + t * MSl : q0 + (t + 1) * MSl],
                        start=True,
                        stop=True,
                    )
            return pss

        # software pipeline: emit QK two chunks ahead of AV so the PE always
        # has independent work queued when an AV blocks on its exp
        pss_q = [emit_qk(0), emit_qk(1)]
        for j in range(NJ):
            pss = pss_q.pop(0)
            pbs = []
            for a in range(2):
                pb = probs_pool.tile(
                    [P, qw], vp.dtype, tag="pb", name=f"pb{un}{j}{a}"
                )
                for t in range(NTl):
                    sub = pb[:, t * MSl : (t + 1) * MSl]
                    if (exp_state[0] * dve_num) % dve_den < dve_num:
                        # Schraudolph exp on the Vector engine: one mult+add
                        # pass, fp32 PSUM in, int16 store whose bits are the
                        # bf16 probs
                        nc.vector.tensor_scalar(
                            sub.bitcast(mybir.dt.int16),
                            pss[a][t],
                            EXP_A,
                            EXP_B,
                            mybir.AluOpType.mult,
                            mybir.AluOpType.add,
                        )
                    else:
                        nc.scalar.activation(
                            sub, pss[a][t], mybir.ActivationFunctionType.Exp
                        )
                    exp_state[0] += 1
                pbs.append(pb)
            if j + 2 < NJ:
                pss_q.append(emit_qk(j + 2))
            for a in range(2):
                for t in range(NTl):
                    nc.tensor.matmul(
                        po[a][:, t * MSl : (t + 1) * MSl],
                        lhsT=vps[2 * p + a][:, j, :],
                        rhs=pbs[a][:, t * MSl : (t + 1) * MSl],
                        start=(j == 0),
                        stop=(j == NJ - 1),
                    )
        # Evacuation: plain Scalar copies free the po banks quickly (the
        # exp engines' in-order queues must never wait on the normalize
        # chain -- a slow op there stalls all subsequent exp and thus AV).
        # Normalization then runs entirely off the exp engines: the
        # denominator rows round-trip through DRAM into a partition-parallel
        # layout, gpsimd computes the final normalize multiplies (SBUF-only).
        dn = work.tile([65, qw], F32, tag="dn", name=f"dn{un}")
        nc.scalar.copy(dn[64:65, :], po[0][64:65, :])
        nc.scalar.copy(dn[32:33, :], po[1][32:33, :])
        xu = work.tile([P, qw], F32, tag="xu", name=f"xu{un}")
        nc.scalar.copy(xu[0:64, :], po[0][0:64, :])
        nc.scalar.copy(xu[64:128, :], po[1][64:128, :])
        dsc = dram.tile([2, qw], F32, tag="dsc", name=f"dsc{un}")
        nc.sync.dma_start(dsc[0:1, :], dn[64:65, :])
        nc.sync.dma_start(dsc[1:2, :], dn[32:33, :])
        G = 2 * qw // P
        dnb = work.tile([P, G], F32, tag="dnb", name=f"dnb{un}")
        nc.sync.dma_start(
            dnb,
            bass.AP(tensor=dsc.tensor, offset=dsc.offset, ap=[[G, P], [1, G]]),
        )
        rcb = work.tile([P, G], F32, tag="rcb", name=f"rcb{un}")
        nc.vector.reciprocal(rcb, dnb)
        dsc2 = dram.tile([2, qw], F32, tag="dsc2", name=f"dsc2{un}")
        nc.sync.dma_start(
            bass.AP(tensor=dsc2.tensor, offset=dsc2.offset,
                    ap=[[G, P], [1, G]]),
            rcb,
        )
        rbd = work.tile([P, qw], F32, tag="rbd", name=f"rbd{un}")
        nc.sync.dma_start(rbd[0:64, :], _bcast_row(dsc2[0], 64))
        nc.sync.dma_start(rbd[64:128, :], _bcast_row(dsc2[1], 64))
        nc.gpsimd.tensor_mul(
            xts[p][0:64, q0 : q0 + qw], xu[0:64, :], rbd[0:64, :]
        )
        nc.gpsimd.tensor_mul(
            xts[p][64:128, q0 : q0 + qw], xu[64:128, :], rbd[64:128, :]
        )
        if nburst:
            warm_burst(nburst, f"rw{un}")

    # 512-query units throughout: score tiles are then 1 psum bank each, so
    # with bufs=6 three chunks of scores are in flight and the
    # QK(j+1) -> exp(j) psum-reuse loop (~2us of cross-engine latency) spans
    # 3 chunk periods instead of pacing every chunk.  The projection for each
    # query quarter is emitted one unit after both pairs finish it, so every
    # normalize chain overlaps another unit's attention.
    QW = 512
    nq = n // QW  # query quarters
    for qi in range(nq):
        for p in range(npair):
            attention_unit(p, qi * QW, QW, f"u{qi}{p}")
        if qi > 0:
            proj_tiles((qi - 1) * (QW // P), qi * (QW // P))
    proj_tiles((nq - 1) * (QW // P), n // P)


def build_module(n=N, ow=C, npair=NPAIR):
    qkd, avd, pjd = _mm_dtypes()
    nc = bacc.Bacc("TRN2", target_bir_lowering=False, debug=False, num_devices=NCORES)
    nj = n // 128
    qt = nc.dram_tensor("qt", [npair, 128, n], qkd, kind="ExternalInput")
    kt = nc.dram_tensor("kt", [npair, 128, n], qkd, kind="ExternalInput")
    vp = nc.dram_tensor("vp", [2 * npair, 128, nj, 128], avd, kind="ExternalInput")
    wt = nc.dram_tensor("wt", [npair, 128, ow], pjd, kind="ExternalInput")
    out = nc.dram_tensor("out", [n, ow], _out_dtype(), kind="ExternalOutput")
    with tile.TileContext(nc) as tc:
        attention_body(tc, out.ap(), qt.ap(), kt.ap(), vp.ap(), wt.ap())
    nc.compile()
    return nc


def shard_inputs(q, k, v, proj_w):
    """Build the 8 per-core input maps from the full tensors."""
    q = np.asarray(q, dtype=np.float32)
    k = np.asarray(k, dtype=np.float32)
    v = np.asarray(v, dtype=np.float32)
    proj_w = np.asarray(proj_w, dtype=np.float32)
    b_, n_, c_ = q.shape
    h_ = k.shape[1]
    d_ = c_ // h_
    nj = n_ // 128
    _np_dt = {"f32": np.float32, "f32r": np.float32, "bf16": ml_dtypes.bfloat16,
              "f16": np.float16}
    qk_np = _np_dt[os.environ.get("ATTN_KERNEL_QK_DT", "f16")]
    # [B, H, D, N]
    qh = np.ascontiguousarray(
        q.reshape(b_, n_, h_, d_).transpose(0, 2, 3, 1).astype(qk_np)
    )
    kh = np.ascontiguousarray(k.transpose(0, 1, 3, 2).astype(qk_np))
    in_maps = []
    for c in range(NCORES):
        b = c // 4
        hh0 = HPC * (c % 4)
        qt = np.ascontiguousarray(qh[b, hh0 : hh0 + HPC].reshape(NPAIR, 128, n_))
        kt = np.ascontiguousarray(kh[b, hh0 : hh0 + HPC].reshape(NPAIR, 128, n_))
        avd = os.environ.get("ATTN_KERNEL_AV_DT", "bf16")
        vp_np = ml_dtypes.bfloat16 if avd == "bf16" else np.float32
        vp = np.zeros((HPC, 128, nj, 128), vp_np)
        for hh in range(HPC):
            vv = v[b, hh0 + hh].reshape(nj, 128, d_).transpose(1, 0, 2)
            if hh % 2 == 0:
                vp[hh][:, :, 0:64] = vv
                vp[hh][:, :, 64] = 1.0
            else:
                vp[hh][:, :, 64:128] = vv
                vp[hh][:, :, 32] = 1.0
        ch0 = hh0 * d_
        pj_np = _np_dt[os.environ.get("ATTN_KERNEL_PJ_DT", "f16")]
        wt = np.ascontiguousarray(
            proj_w[:, ch0 : ch0 + HPC * d_].T.reshape(NPAIR, 128, c_).astype(pj_np)
        )
        in_maps.append({"qt": qt, "kt": kt, "vp": vp, "wt": wt})
    return in_maps


def reduce_outputs(results, proj_b):
    """Sum the per-core partial projections per batch and add the bias."""
    outs = [np.asarray(r["out"], dtype=np.float32) for r in results]
    full = np.stack(
        [outs[0] + outs[1] + outs[2] + outs[3], outs[4] + outs[5] + outs[6] + outs[7]]
    )
    return (full + np.asarray(proj_b, dtype=np.float32)[None, None, :]).astype(
        np.float32
    )


_NC_CACHE = {}


def _get_module():
    if "nc" not in _NC_CACHE:
        _NC_CACHE["nc"] = build_module()
    return _NC_CACHE["nc"]


def kernel(q, k, v, proj_w, proj_b):
    nc = _get_module()
    in_maps = shard_inputs(q, k, v, proj_w)
    trace = bool(int(os.environ.get("ATTN_KERNEL_TRACE", "0")))
    kwargs = {}
    tmpdir = os.environ.get("ATTN_KERNEL_TMPDIR")
    if trace and tmpdir:
        os.makedirs(tmpdir, exist_ok=True)
        kwargs["tmpdir"] = tmpdir
    res = bass_utils.run_bass_kernel_spmd(
        nc, in_maps, core_ids=list(range(NCORES)), trace=trace, **kwargs
    )
    if trace:
        _NC_CACHE["last_results"] = res
    return reduce_outputs(res.results, proj_b)
